# revision 26
# baseline (speedup 1.0000x reference)
"""Trainium2 Bass kernel: full (non-causal) softmax attention.

Input:  query/key/value [1, 4096, 16, 128] f32 (B, S, H, D).
Output: [1, 4096, 16, 128] f32 = softmax(Q K^T / sqrt(D)) V per head.

Sharding: 16 heads over 8 cores -> 2 heads per core, no collectives.
Host pre-transposes Q,K per head to [D, S] and casts to bf16; the device
returns the UN-normalized attention output transposed [D, S] fp32 plus
the softmax denominator row [S]; the host does the final divide.

Device pipeline, per head, per query-chunk QC (1024 queries):
  for kt in 32 key-chunks (128 keys each):
    ST[kt] = scores^T chunk: psum[128k, QCq] fp32  (two N=512 bf16 matmuls,
             stationary KT chunk; moving operand = QT columns)
    PT[kt] = exp(ST / sqrt(128)) -> SBUF bf16     (ACT, psum->sbuf)
    OUT   += V_kt^T @ PT[kt]                      (bf16 matmuls, fp32 psum)
    den   += colsum(PT[kt]): mostly PE ones-matmuls accumulating into a
             psum row (fp32-exact); first few chunks go through DVE bf16
             adds so the PE never stalls on the previous chunk's epilogue.
ACT (exp) is the throughput floor (~1337ns per [128,1024] tile); bf16
halves the PE streaming cost so everything else hides below it.
"""

import os
import sys
from contextlib import ExitStack

import ml_dtypes
import numpy as np

sys.path.insert(0, "/opt/trn_rl_repo")

import concourse.bacc as bacc
import concourse.bass as bass
import concourse.tile as tile
from concourse import mybir
from concourse.bass_utils import run_bass_kernel_spmd

N_CORES = 8
S = 4096
H = 16
D = 128
HEADS_PER_CORE = H // N_CORES  # 2
KT_CHUNK = 128                  # keys per score tile (psum partition dim)
QC = 1024                       # queries per super-chunk (ACT tile free dim)
NMM = 512                       # moving free dim per matmul (psum bank fp32)
SCALE = float(D) ** -0.5

F32 = mybir.dt.float32
BF16 = mybir.dt.bfloat16

# denominator reduction: all kt-chunks accumulate through DVE bf16 adds
# (PE has no headroom under the ACT exp floor); a final pair of
# ones-matmuls reduces the [128, QC] partial-sum tile into the psum den
# row.


def build_program(s=S, heads=HEADS_PER_CORE):
    nc = bacc.Bacc("TRN2", target_bir_lowering=False, debug=False,
                   num_devices=N_CORES)

    n_kt = s // KT_CHUNK
    n_qc = s // QC

    qt_d = nc.dram_tensor("qt", [heads, D, s], BF16, kind="ExternalInput")
    kt_d = nc.dram_tensor("kt", [heads, D, s], BF16, kind="ExternalInput")
    v_d = nc.dram_tensor("v", [heads, s, D], BF16, kind="ExternalInput")
    out_d = nc.dram_tensor("out", [heads, D, s], F32, kind="ExternalOutput")
    # per-query-chunk key-partial sums; host reduces over the 128
    # partition rows to get the softmax denominator.
    acc_d = nc.dram_tensor("acc", [heads, s // QC, 128, QC], BF16,
                           kind="ExternalOutput")

    with tile.TileContext(nc) as tc, ExitStack() as ctx:
        qkv_pool = ctx.enter_context(tc.tile_pool(name="qkv", bufs=2))
        pt_pool = ctx.enter_context(tc.tile_pool(name="pt", bufs=8))
        acc_pool = ctx.enter_context(tc.tile_pool(name="acc", bufs=3))
        osb_pool = ctx.enter_context(tc.tile_pool(name="osb", bufs=3))
        st_pool = ctx.enter_context(
            tc.tile_pool(name="st", bufs=3, space="PSUM"))
        outp_pool = ctx.enter_context(
            tc.tile_pool(name="outp", bufs=2, space="PSUM"))
        warm_pool = ctx.enter_context(tc.tile_pool(name="warm", bufs=1))

        # Warm the ACT exp table-set immediately so the ~2.7us
        # ACT_TABLE_LOAD overlaps the initial input DMA instead of
        # serializing before the first real exp. Also prewarm the DMA
        # queue with a tiny transfer so descriptor-path startup latency
        # is paid before the real input loads.
        warm = warm_pool.tile([128, 1], BF16, tag="warm")
        warm_f = warm_pool.tile([128, 1], F32, tag="warm_f")
        nc.vector.memset(warm_f[:], 0.0)
        nc.scalar.activation(warm[:], warm_f[:],
                             mybir.ActivationFunctionType.Exp)

        # Loads are split into pieces with one TILE per piece (dependency
        # tracking is whole-tile, so separate tiles are what actually
        # lets early QK matmuls start before the full head has landed).
        # Head 0 gets a tiny 1-chunk first piece: the first QK needs just
        # 32KB of K, so the exp stream starts ~4us sooner.
        PC = s // n_qc       # queries per qt piece == QC

        def load_head(h):
            sizes = [1, 7, 8, 8, 8] if h == 0 else [16, 16]
            qt_p, kt_p, v_p = [], [], []
            bounds = []
            c0 = 0
            for i, nch in enumerate(sizes):
                sl = slice(c0 * KT_CHUNK, (c0 + nch) * KT_CHUNK)
                kt_sb = qkv_pool.tile([D, nch * KT_CHUNK], BF16,
                                      tag=f"kt{h}_{i}", name=f"kt{h}_{i}")
                nc.sync.dma_start(out=kt_sb[:], in_=kt_d[h][:, sl])
                if i == 0:
                    qt_sb = qkv_pool.tile([D, PC], BF16, tag=f"qt{h}_0",
                                          name=f"qt{h}_0")
                    nc.sync.dma_start(out=qt_sb[:], in_=qt_d[h][:, 0:PC])
                    qt_p.append(qt_sb)
                v_sb = qkv_pool.tile([128, nch, D], BF16,
                                     tag=f"v{h}_{i}", name=f"v{h}_{i}")
                nc.sync.dma_start(
                    out=v_sb[:],
                    in_=v_d[h][sl].rearrange("(c p) d -> p c d", p=128))
                kt_p.append((c0, kt_sb))
                v_p.append((c0, v_sb))
                bounds.append(c0)
                c0 += nch
            for p in range(1, n_qc):
                sl = slice(p * PC, (p + 1) * PC)
                qt_sb = qkv_pool.tile([D, PC], BF16, tag=f"qt{h}_{p}",
                                      name=f"qt{h}_{p}")
                nc.sync.dma_start(out=qt_sb[:], in_=qt_d[h][:, sl])
                qt_p.append(qt_sb)

            return qt_p, kt_p, v_p

        def piece_lookup(pieces, kt):
            for c0, t in reversed(pieces):
                if kt >= c0:
                    return t, kt - c0
            raise AssertionError

        heads_sb = [load_head(0)]

        # Deferred epilogue work, interleaved into the next chunk's matmul
        # stream so the PE pipeline never waits on DVE.
        pending = []

        assert QC == PC
        for h in range(heads):
            qt_p, kt_p, v_p = heads_sb[h]
            if h + 1 < heads:
                heads_sb.append(load_head(h + 1))
            for qc in range(n_qc):
                q0 = qc * QC
                qt_sb = qt_p[qc]
                out_ps = [outp_pool.tile([D, NMM], F32, tag="outp",
                                         name=f"out_ps{j}")
                          for j in range(QC // NMM)]
                accs = []
                pt0 = None
                for kt in range(n_kt):
                    st = st_pool.tile([128, QC], F32, tag="st")
                    kt_t, kloc = piece_lookup(kt_p, kt)
                    lhs_k = kt_t[:, kloc * KT_CHUNK:(kloc + 1) * KT_CHUNK]
                    for j in range(QC // NMM):
                        nc.tensor.matmul(
                            st[:, j * NMM:(j + 1) * NMM],
                            lhs_k,
                            qt_sb[:, j * NMM:(j + 1) * NMM],
                            start=True, stop=True)
                    pt = pt_pool.tile([128, QC], BF16, tag="pt")
                    nc.scalar.activation(
                        pt[:], st[:], mybir.ActivationFunctionType.Exp,
                        scale=SCALE)
                    v_t, vloc = piece_lookup(v_p, kt)
                    lhs_v = v_t[:, vloc, :]
                    for j in range(QC // NMM):
                        nc.tensor.matmul(
                            out_ps[j][:],
                            lhs_v,
                            pt[:, j * NMM:(j + 1) * NMM],
                            start=(kt == 0), stop=(kt == n_kt - 1))
                    # softmax denominator partial reduction (DVE bf16).
                    if kt == 0:
                        pt0 = pt
                    elif kt == 1:
                        a = acc_pool.tile([128, QC], BF16, tag="accV",
                                          name="accV")
                        nc.vector.tensor_add(a[:], pt0[:], pt[:])
                        accs.append(a)
                    else:
                        b = acc_pool.tile([128, QC], BF16, tag="accVb",
                                          name="accVb")
                        nc.vector.tensor_add(b[:], accs[-1][:], pt[:])
                        accs.append(b)
                    if pending:
                        pending.pop(0)()

                def finish(out_ps=out_ps, accs=accs, h=h, qc=qc, q0=q0):
                    # one tile per 512-column piece: whole-tile dependency
                    # tracking then lets each piece's DMA start right
                    # after its own copy.
                    out_sb = [osb_pool.tile([D, NMM], F32, tag=f"out_sb{j}",
                                            name=f"out_sb{j}")
                              for j in range(QC // NMM)]
                    acc = accs[-1]

                    def s1():
                        nc.sync.dma_start(out=acc_d[h, qc], in_=acc[:])

                    def s2():
                        for j in range(QC // NMM):
                            nc.vector.tensor_copy(out_sb[j][:], out_ps[j][:])
                            nc.sync.dma_start(
                                out=out_d[h][:, q0 + j * NMM:
                                             q0 + (j + 1) * NMM],
                                in_=out_sb[j][:])

                    return [s1, s2]

                pending.extend(finish())
        while pending:
            pending.pop(0)()

    nc.compile()
    return nc


def _install_ntff_hook():
    """Provide antenv.axon_hooks (absent in this image) so that
    run_bass_kernel_spmd(trace=True) can capture NTFF profiles via the
    axon .so — mirrors trn_agent_boot.trn_boot._ntff_profile_via_ctypes."""
    try:
        from antenv.axon_hooks import get_axon_ntff_profile_hook  # noqa: F401
        return
    except ImportError:
        pass
    import contextlib
    import ctypes
    import types

    so_path = "/opt/axon/libaxon_pjrt.so"
    lib = ctypes.CDLL(so_path)
    if not hasattr(lib, "axon_start_nrt_profile"):
        return
    lib.axon_start_nrt_profile.argtypes = [
        ctypes.POINTER(ctypes.c_int64), ctypes.c_size_t]
    lib.axon_start_nrt_profile.restype = ctypes.c_int64
    lib.axon_stop_nrt_profile.argtypes = [ctypes.c_char_p]
    lib.axon_stop_nrt_profile.restype = ctypes.c_int64

    @contextlib.contextmanager
    def _hook(output_dir, device_ids):
        import jax
        jax.devices()
        if device_ids:
            ids = (ctypes.c_int64 * len(device_ids))(*device_ids)
            rc = lib.axon_start_nrt_profile(ids, len(device_ids))
        else:
            rc = lib.axon_start_nrt_profile(None, 0)
        if rc != 0:
            raise RuntimeError(f"axon_start_nrt_profile rc={rc}")
        try:
            yield
        finally:
            n = lib.axon_stop_nrt_profile(str(output_dir).encode())
            print(f"ntff profile: {n} file(s) written to {output_dir}")

    mod = types.ModuleType("antenv.axon_hooks")
    mod.get_axon_ntff_profile_hook = lambda: _hook
    mod.set_axon_ntff_profile_hook = lambda h: None
    import antenv
    sys.modules["antenv.axon_hooks"] = mod
    antenv.axon_hooks = mod


_CACHE = {}


def _get_program():
    key = "main"
    if key not in _CACHE:
        _CACHE[key] = build_program()
    return _CACHE[key]


def kernel(query, key, value, trace=False, **trace_kwargs):
    assert query.shape == (1, S, H, D)
    nc = _get_program()

    bf16 = ml_dtypes.bfloat16
    q = np.asarray(query, dtype=np.float32)[0]   # [S, H, D]
    k = np.asarray(key, dtype=np.float32)[0]
    v = np.asarray(value, dtype=np.float32)[0]

    in_maps = []
    for c in range(N_CORES):
        hs = slice(c * HEADS_PER_CORE, (c + 1) * HEADS_PER_CORE)
        # [S, h, D] -> [h, D, S]
        qt = np.ascontiguousarray(
            q[:, hs, :].transpose(1, 2, 0)).astype(bf16)
        kt = np.ascontiguousarray(
            k[:, hs, :].transpose(1, 2, 0)).astype(bf16)
        vv = np.ascontiguousarray(
            v[:, hs, :].transpose(1, 0, 2)).astype(bf16)
        in_maps.append({"qt": qt, "kt": kt, "v": vv})

    if trace:
        _install_ntff_hook()
    res = run_bass_kernel_spmd(nc, in_maps, core_ids=list(range(N_CORES)),
                               trace=trace, **trace_kwargs)

    out = np.empty((1, S, H, D), dtype=np.float32)
    for c in range(N_CORES):
        o = res.results[c]["out"]    # [h, D, S] unnormalized
        acc = np.asarray(res.results[c]["acc"], dtype=np.float32)
        den = acc.sum(axis=2).reshape(HEADS_PER_CORE, S)  # [h, S]
        for i in range(HEADS_PER_CORE):
            out[0, :, c * HEADS_PER_CORE + i, :] = (o[i] / den[i][None, :]).T
    if trace:
        kernel.last_results = res
    return out


# revision 29
# speedup vs baseline: 1.0020x; 1.0020x over previous
"""Trainium2 Bass kernel: full (non-causal) softmax attention.

Input:  query/key/value [1, 4096, 16, 128] f32 (B, S, H, D).
Output: [1, 4096, 16, 128] f32 = softmax(Q K^T / sqrt(D)) V per head.

Sharding: 16 heads over 8 cores -> 2 heads per core, no collectives.
Host pre-transposes Q,K per head to [D, S] and casts to bf16; the device
returns the UN-normalized attention output transposed [D, S] fp32 plus
bf16 key-partial-sum tiles; the host reduces those over the 128
partition rows for the softmax denominator and does the final divide.

Device pipeline, per head, per query-chunk QC (1024 queries):
  for kt in 32 key-chunks (128 keys each):
    ST[kt] = scores^T chunk: psum[128k, QCq] fp32  (two N=512 bf16 matmuls,
             stationary KT chunk; moving operand = QT columns)
    PT[kt] = exp(ST / sqrt(128)) -> SBUF bf16     (ACT, psum->sbuf)
    OUT   += V_kt^T @ PT[kt]                      (bf16 matmuls, fp32 psum)
    acc   += PT[kt]                               (DVE bf16 2x-mode adds)

Steady state is an ACT(exp)-bound software pipeline at ~1.01us per
[128,1024] score tile — the scalar engine's intrinsic (N + ~180)/1.2GHz
with a continuously-full queue. Everything else is sized to hide under
it: PE ~1.02us/chunk (4 bf16 N=512 matmuls + 2 weight loads; bf16
moving operands stream 2x faster than fp32, and the whole denominator
reduction is off the PE), DVE ~0.72us/chunk. PSUM is exactly full:
3 score buffers (6 banks, the 3rd absorbs semaphore jitter when the
chip P0-downclocks the PE to ~2GHz) + 2 output accumulators (2 banks).
The exp table-set is warmed at t=0 and the first K piece is a single
128-key chunk so the exp stream starts ~11us in (framework preamble is
~8us); separate TILES per load piece are required for that — hazard
tracking is whole-tile, sub-slices do not help.
"""

import os
import sys
from contextlib import ExitStack

import ml_dtypes
import numpy as np

sys.path.insert(0, "/opt/trn_rl_repo")

import concourse.bacc as bacc
import concourse.bass as bass
import concourse.tile as tile
from concourse import mybir
from concourse.bass_utils import run_bass_kernel_spmd

N_CORES = 8
S = 4096
H = 16
D = 128
HEADS_PER_CORE = H // N_CORES  # 2
KT_CHUNK = 128                  # keys per score tile (psum partition dim)
QC = 1024                       # queries per super-chunk (ACT tile free dim)
NMM = 512                       # moving free dim per matmul (psum bank fp32)
SCALE = float(D) ** -0.5

F32 = mybir.dt.float32
BF16 = mybir.dt.bfloat16

# denominator reduction: all kt-chunks accumulate through DVE bf16 adds
# (PE has no headroom under the ACT exp floor); a final pair of
# ones-matmuls reduces the [128, QC] partial-sum tile into the psum den
# row.


def build_program(s=S, heads=HEADS_PER_CORE):
    nc = bacc.Bacc("TRN2", target_bir_lowering=False, debug=False,
                   num_devices=N_CORES)

    n_kt = s // KT_CHUNK
    n_qc = s // QC

    qt_d = nc.dram_tensor("qt", [heads, D, s], BF16, kind="ExternalInput")
    kt_d = nc.dram_tensor("kt", [heads, D, s], BF16, kind="ExternalInput")
    v_d = nc.dram_tensor("v", [heads, s, D], BF16, kind="ExternalInput")
    out_d = nc.dram_tensor("out", [heads, D, s], F32, kind="ExternalOutput")
    # per-query-chunk key-partial sums; host reduces over the 128
    # partition rows to get the softmax denominator.
    acc_d = nc.dram_tensor("acc", [heads, s // QC, 128, QC], BF16,
                           kind="ExternalOutput")

    with tile.TileContext(nc) as tc, ExitStack() as ctx:
        qkv_pool = ctx.enter_context(tc.tile_pool(name="qkv", bufs=2))
        pt_pool = ctx.enter_context(tc.tile_pool(name="pt", bufs=8))
        acc_pool = ctx.enter_context(tc.tile_pool(name="acc", bufs=3))
        osb_pool = ctx.enter_context(tc.tile_pool(name="osb", bufs=3))
        st_pool = ctx.enter_context(
            tc.tile_pool(name="st", bufs=3, space="PSUM"))
        outp_pool = ctx.enter_context(
            tc.tile_pool(name="outp", bufs=2, space="PSUM"))
        warm_pool = ctx.enter_context(tc.tile_pool(name="warm", bufs=1))

        # Warm the ACT exp table-set immediately so the ~2.7us
        # ACT_TABLE_LOAD overlaps the initial input DMA instead of
        # serializing before the first real exp. Also prewarm the DMA
        # queue with a tiny transfer so descriptor-path startup latency
        # is paid before the real input loads.
        warm = warm_pool.tile([128, 1], BF16, tag="warm")
        warm_f = warm_pool.tile([128, 1], F32, tag="warm_f")
        nc.vector.memset(warm_f[:], 0.0)
        nc.scalar.activation(warm[:], warm_f[:],
                             mybir.ActivationFunctionType.Exp)

        # Loads are split into pieces with one TILE per piece (dependency
        # tracking is whole-tile, so separate tiles are what actually
        # lets early QK matmuls start before the full head has landed).
        # Head 0 gets a tiny 1-chunk first piece: the first QK needs just
        # 32KB of K, so the exp stream starts ~4us sooner.
        PC = s // n_qc       # queries per qt piece == QC

        def load_head(h):
            sizes = [1, 7, 8, 8, 8] if h == 0 else [16, 16]
            qt_p, kt_p, v_p = [], [], []
            c0 = 0
            for i, nch in enumerate(sizes):
                sl = slice(c0 * KT_CHUNK, (c0 + nch) * KT_CHUNK)
                kt_sb = qkv_pool.tile([D, nch * KT_CHUNK], BF16,
                                      tag=f"kt{h}_{i}", name=f"kt{h}_{i}")
                nc.sync.dma_start(out=kt_sb[:], in_=kt_d[h][:, sl])
                if i == 0:
                    qt_sb = qkv_pool.tile([D, PC], BF16, tag=f"qt{h}_0",
                                          name=f"qt{h}_0")
                    nc.sync.dma_start(out=qt_sb[:], in_=qt_d[h][:, 0:PC])
                    qt_p.append(qt_sb)
                v_sb = qkv_pool.tile([128, nch, D], BF16,
                                     tag=f"v{h}_{i}", name=f"v{h}_{i}")
                nc.sync.dma_start(
                    out=v_sb[:],
                    in_=v_d[h][sl].rearrange("(c p) d -> p c d", p=128))
                kt_p.append((c0, kt_sb))
                v_p.append((c0, v_sb))
                c0 += nch
            for p in range(1, n_qc):
                sl = slice(p * PC, (p + 1) * PC)
                qt_sb = qkv_pool.tile([D, PC], BF16, tag=f"qt{h}_{p}",
                                      name=f"qt{h}_{p}")
                nc.sync.dma_start(out=qt_sb[:], in_=qt_d[h][:, sl])
                qt_p.append(qt_sb)

            return qt_p, kt_p, v_p

        def piece_lookup(pieces, kt):
            for c0, t in reversed(pieces):
                if kt >= c0:
                    return t, kt - c0
            raise AssertionError

        heads_sb = [load_head(0)]

        # Deferred epilogue work, interleaved into the next chunk's matmul
        # stream so the PE pipeline never waits on DVE.
        pending = []

        assert QC == PC
        for h in range(heads):
            qt_p, kt_p, v_p = heads_sb[h]
            if h + 1 < heads:
                heads_sb.append(load_head(h + 1))
            for qc in range(n_qc):
                q0 = qc * QC
                qt_sb = qt_p[qc]
                out_ps = [outp_pool.tile([D, NMM], F32, tag="outp",
                                         name=f"out_ps{j}")
                          for j in range(QC // NMM)]
                accs = []
                pt0 = None
                for kt in range(n_kt):
                    st = st_pool.tile([128, QC], F32, tag="st")
                    kt_t, kloc = piece_lookup(kt_p, kt)
                    lhs_k = kt_t[:, kloc * KT_CHUNK:(kloc + 1) * KT_CHUNK]
                    for j in range(QC // NMM):
                        nc.tensor.matmul(
                            st[:, j * NMM:(j + 1) * NMM],
                            lhs_k,
                            qt_sb[:, j * NMM:(j + 1) * NMM],
                            start=True, stop=True)
                    pt = pt_pool.tile([128, QC], BF16, tag="pt")
                    nc.scalar.activation(
                        pt[:], st[:], mybir.ActivationFunctionType.Exp,
                        scale=SCALE)
                    v_t, vloc = piece_lookup(v_p, kt)
                    lhs_v = v_t[:, vloc, :]
                    for j in range(QC // NMM):
                        nc.tensor.matmul(
                            out_ps[j][:],
                            lhs_v,
                            pt[:, j * NMM:(j + 1) * NMM],
                            start=(kt == 0), stop=(kt == n_kt - 1))
                    # softmax denominator partial reduction (DVE bf16).
                    if kt == 0:
                        pt0 = pt
                    elif kt == 1:
                        a = acc_pool.tile([128, QC], BF16, tag="accV",
                                          name="accV")
                        nc.vector.tensor_add(a[:], pt0[:], pt[:])
                        accs.append(a)
                    else:
                        b = acc_pool.tile([128, QC], BF16, tag="accVb",
                                          name="accVb")
                        nc.vector.tensor_add(b[:], accs[-1][:], pt[:])
                        accs.append(b)
                    if pending:
                        pending.pop(0)()

                def finish(out_ps=out_ps, accs=accs, h=h, qc=qc, q0=q0):
                    # one tile per 512-column piece: whole-tile dependency
                    # tracking then lets each piece's DMA start right
                    # after its own copy.
                    out_sb = [osb_pool.tile([D, NMM], F32, tag=f"out_sb{j}",
                                            name=f"out_sb{j}")
                              for j in range(QC // NMM)]
                    acc = accs[-1]

                    def s1():
                        nc.sync.dma_start(out=acc_d[h, qc], in_=acc[:])

                    def s2():
                        for j in range(QC // NMM):
                            nc.vector.tensor_copy(out_sb[j][:], out_ps[j][:])
                            nc.sync.dma_start(
                                out=out_d[h][:, q0 + j * NMM:
                                             q0 + (j + 1) * NMM],
                                in_=out_sb[j][:])

                    return [s1, s2]

                pending.extend(finish())
        while pending:
            pending.pop(0)()

    nc.compile()
    return nc


def _install_ntff_hook():
    """Provide antenv.axon_hooks (absent in this image) so that
    run_bass_kernel_spmd(trace=True) can capture NTFF profiles via the
    axon .so — mirrors trn_agent_boot.trn_boot._ntff_profile_via_ctypes."""
    try:
        from antenv.axon_hooks import get_axon_ntff_profile_hook  # noqa: F401
        return
    except ImportError:
        pass
    import contextlib
    import ctypes
    import types

    so_path = "/opt/axon/libaxon_pjrt.so"
    lib = ctypes.CDLL(so_path)
    if not hasattr(lib, "axon_start_nrt_profile"):
        return
    lib.axon_start_nrt_profile.argtypes = [
        ctypes.POINTER(ctypes.c_int64), ctypes.c_size_t]
    lib.axon_start_nrt_profile.restype = ctypes.c_int64
    lib.axon_stop_nrt_profile.argtypes = [ctypes.c_char_p]
    lib.axon_stop_nrt_profile.restype = ctypes.c_int64

    @contextlib.contextmanager
    def _hook(output_dir, device_ids):
        import jax
        jax.devices()
        if device_ids:
            ids = (ctypes.c_int64 * len(device_ids))(*device_ids)
            rc = lib.axon_start_nrt_profile(ids, len(device_ids))
        else:
            rc = lib.axon_start_nrt_profile(None, 0)
        if rc != 0:
            raise RuntimeError(f"axon_start_nrt_profile rc={rc}")
        try:
            yield
        finally:
            n = lib.axon_stop_nrt_profile(str(output_dir).encode())
            print(f"ntff profile: {n} file(s) written to {output_dir}")

    mod = types.ModuleType("antenv.axon_hooks")
    mod.get_axon_ntff_profile_hook = lambda: _hook
    mod.set_axon_ntff_profile_hook = lambda h: None
    import antenv
    sys.modules["antenv.axon_hooks"] = mod
    antenv.axon_hooks = mod


_CACHE = {}


def _get_program():
    key = "main"
    if key not in _CACHE:
        _CACHE[key] = build_program()
    return _CACHE[key]


def kernel(query, key, value, trace=False, **trace_kwargs):
    assert query.shape == (1, S, H, D)
    nc = _get_program()

    bf16 = ml_dtypes.bfloat16
    q = np.asarray(query, dtype=np.float32)[0]   # [S, H, D]
    k = np.asarray(key, dtype=np.float32)[0]
    v = np.asarray(value, dtype=np.float32)[0]

    in_maps = []
    for c in range(N_CORES):
        hs = slice(c * HEADS_PER_CORE, (c + 1) * HEADS_PER_CORE)
        # [S, h, D] -> [h, D, S]
        qt = np.ascontiguousarray(
            q[:, hs, :].transpose(1, 2, 0)).astype(bf16)
        kt = np.ascontiguousarray(
            k[:, hs, :].transpose(1, 2, 0)).astype(bf16)
        vv = np.ascontiguousarray(
            v[:, hs, :].transpose(1, 0, 2)).astype(bf16)
        in_maps.append({"qt": qt, "kt": kt, "v": vv})

    if trace:
        _install_ntff_hook()
    res = run_bass_kernel_spmd(nc, in_maps, core_ids=list(range(N_CORES)),
                               trace=trace, **trace_kwargs)

    out = np.empty((1, S, H, D), dtype=np.float32)
    for c in range(N_CORES):
        o = res.results[c]["out"]    # [h, D, S] unnormalized
        acc = np.asarray(res.results[c]["acc"], dtype=np.float32)
        den = acc.sum(axis=2).reshape(HEADS_PER_CORE, S)  # [h, S]
        for i in range(HEADS_PER_CORE):
            out[0, :, c * HEADS_PER_CORE + i, :] = (o[i] / den[i][None, :]).T
    if trace:
        kernel.last_results = res
    return out
